# revision 13
# baseline (speedup 1.0000x reference)
"""Trainium2 Bass kernel for nn_H_ATT (GatedTrans pair-attention block).

Math (per example):
  HE = tanh(hist@W_hy+b_hy) * lrelu(hist@W_hg+b_hg)      [R, H]
  QE = tanh(ques@W_qy+b_qy) * lrelu(ques@W_qg+b_qg)      [R, H]
  num[q,h]  = sum_k QE[q,k]*W_att[k]*HE[h,k]
  den[q,h]  = sqrt(sum_k QE[q,k]^2 * HE[h,k]^2)
  s = num / max(den, eps)          (b_att cancels in softmax)
  att = causal_softmax(s)          (softmax*tril/renorm == masked softmax)
  feat = att @ hist                 [R, 2H]

Sharding: pure data parallel, 8 examples per core on 8 NeuronCores.

The embedding GEMMs dominate both PE time and HBM traffic; they run in
fp8(e4m3) with DoubleRow (weights pre-scaled by 256 on the host, descale
fused into the activation's scale argument). The score/att/feat path stays
bf16/f32. All DRAM inputs are host-packed so every DMA line is contiguous
per partition.
"""

import numpy as np
import ml_dtypes

import bass_rust
import concourse.bass as bass
import concourse.mybir as mybir
import concourse.tile as tile
from concourse.vector_clock import ScopedClock

# ---------------------------------------------------------------------------
# Workaround: this walrus build accepts only ONE semaphore wait on an SP
# Drain, but TileContext's tail drain carries one wait per live semaphore.
# Split them across a chain of drains.
# ---------------------------------------------------------------------------


def _patched_drain_and_barrier(self, tick_clock, wait_clock):
    nc = self.nc
    drain_inst = nc.sync.drain()
    wait_clock.add_sem_waits(
        drain_inst.ins, ScopedClock({None: tick_clock.global_clock})
    )
    waits = list(drain_inst.ins.sync_info.on_wait)
    if len(waits) > 1:
        drain_inst.ins.sync_info = bass_rust.SyncInfo(
            on_wait=waits[:1], on_update=list(drain_inst.ins.sync_info.on_update)
        )
        for i in range(1, len(waits)):
            extra = nc.sync.drain()
            extra.ins.sync_info = bass_rust.SyncInfo(
                on_wait=waits[i : i + 1], on_update=[]
            )
    nc.all_engine_barrier()
    assert self.sems is not None
    popped = nc._tile_sem_poison_stack.pop()
    assert popped is self._sem_poison
    nc.clear_and_free_semaphores(list(self.sems.allocated().values()))
    nc.all_engine_barrier()


tile.TileContext._drain_and_barrier = _patched_drain_and_barrier


def _split_multi_waits(nc):
    """This walrus build accepts at most one semaphore wait per instruction.
    Hoist extra waits onto standalone EventSemaphore instructions inserted
    just before the owning instruction in the same engine's stream."""
    uid = [0]
    for f in nc.m.functions:
        for bb in f.blocks:
            out = []
            for inst in bb.instructions:
                si = inst.sync_info
                if si is not None and len(si.on_wait) > 1:
                    waits = list(si.on_wait)
                    for w in waits[:-1]:
                        nop = mybir.InstEventSemaphore(
                            name=f"I-waitsplit-{uid[0]}", ins=[], outs=[]
                        )
                        uid[0] += 1
                        nop.engine = inst.engine
                        nop.sync_info = bass_rust.SyncInfo(
                            on_wait=[w], on_update=[]
                        )
                        out.append(nop)
                    inst.sync_info = bass_rust.SyncInfo(
                        on_wait=[waits[-1]], on_update=list(si.on_update)
                    )
                out.append(inst)
            bb.instructions[:] = out

# ---------------------------------------------------------------------------

B, R, H, IN = 64, 32, 1024, 2048
NCORES = 8
BL = B // NCORES  # examples per core
BR = BL * R  # 256 rows per core
MC = H // 128  # 8 h chunks
NEG = -1.0e30
WSCALE = 256.0  # fp8 weight pre-scale

F32 = mybir.dt.float32
BF16 = mybir.dt.bfloat16
E4 = mybir.dt.float8e4

ACT = mybir.ActivationFunctionType
DR = mybir.MatmulPerfMode.DoubleRow


def build_program(fp8=True):
    """Per-core Bass program. fp8: embedding GEMMs in e4m3 + DoubleRow;
    else bf16 regular matmuls. Everything downstream is identical."""
    xdt = E4 if fp8 else BF16
    KC = 8 if fp8 else 16  # contraction chunks ([128,2] pairs when fp8)
    kshape = [KC, 2, 128] if fp8 else [KC, 128]
    ascale = 1.0 / WSCALE if fp8 else 1.0

    nc = bass.Bass()
    # activations: [128, KC(,2), BR], partition-major contiguous
    qt_d = nc.dram_tensor("qt", [128] + kshape[:-1] + [BR], xdt, kind="ExternalInput")
    ht_d = nc.dram_tensor("ht", [128] + kshape[:-1] + [BR], xdt, kind="ExternalInput")
    # weights: [MC, 128, 2branch, KC(,2), 128]
    wh_d = nc.dram_tensor("wh", [MC, 128, 2] + kshape, xdt, kind="ExternalInput")
    wq_d = nc.dram_tensor("wq", [MC, 128, 2] + kshape, xdt, kind="ExternalInput")
    hn_d = nc.dram_tensor("hn", [128, 2, IN], BF16, kind="ExternalInput")
    b_d = {
        n: nc.dram_tensor(n, [128, MC], F32, kind="ExternalInput")
        for n in ("bhy", "bhg", "bqy", "bqg")
    }
    watt_d = nc.dram_tensor("watt", [128, MC], F32, kind="ExternalInput")
    mask_d = nc.dram_tensor("mask", [128, 128], F32, kind="ExternalInput")
    ident_d = nc.dram_tensor("ident", [128, 128], F32, kind="ExternalInput")
    feat_d = nc.dram_tensor("feat", [2, 128, IN], BF16, kind="ExternalOutput")

    with tile.TileContext(nc) as tc:
        with (
            tc.tile_pool(name="sb", bufs=1) as sb,
            tc.tile_pool(name="wts", bufs=4) as wts,
            tc.tile_pool(name="tmp", bufs=2) as tmp,
        ):
            # acts + consts on the ACT hwdge queue (independent of the
            # weight queue so weight-buffer waits never delay them).
            # qt split in two tiles so the first matmuls start earlier.
            KH = KC // 2
            qta = sb.tile([128, KH] + kshape[1:-1] + [BR], xdt, tag="qta")
            nc.scalar.dma_start(qta[:], qt_d[:, :KH])
            qtb = sb.tile([128, KH] + kshape[1:-1] + [BR], xdt, tag="qtb")
            nc.scalar.dma_start(qtb[:], qt_d[:, KH:])

            def qt(k):
                return qta[:, k] if k < KH else qtb[:, k - KH]

            # tiny consts next (first activations need the biases), then
            # the bulk hist loads (not needed until the hist phase / tail)
            bsb = {}
            for n in ("bqy", "bqg", "bhy", "bhg"):
                bsb[n] = sb.tile([128, MC], F32, name=n, tag=n)
                nc.scalar.dma_start(bsb[n][:], b_d[n][:])
            watt = sb.tile([128, MC], F32, tag="watt")
            nc.scalar.dma_start(watt[:], watt_d[:])

            ht_t = sb.tile([128] + kshape[:-1] + [BR], xdt, tag="ht")
            nc.scalar.dma_start(ht_t[:], ht_d[:])

            def ht(k):
                return ht_t[:, k]

            mask = sb.tile([128, 128], F32, tag="mask")
            nc.scalar.dma_start(mask[:], mask_d[:])
            ident = sb.tile([128, 128], F32, tag="ident")
            nc.scalar.dma_start(ident[:], ident_d[:])
            hn = sb.tile([128, 2, IN], BF16, tag="hn")
            nc.scalar.dma_start(hn[:], hn_d[:])

            he = sb.tile([128, MC, BR], BF16, tag="he")
            he2 = sb.tile([128, MC, BR], BF16, tag="he2")
            qew = sb.tile([128, MC, BR], BF16, tag="qew")
            qe2 = sb.tile([128, MC, BR], BF16, tag="qe2")

            with (
                tc.tile_pool(name="pse", bufs=2, space="PSUM") as pse,
                tc.tile_pool(name="psnd", bufs=1, space="PSUM") as psnd,
            ):
                num_ps = [
                    psnd.tile([128, 128], F32, name=f"num{g}", tag=f"num{g}")
                    for g in range(2)
                ]
                den_ps = [
                    psnd.tile([128, 128], F32, name=f"den{g}", tag=f"den{g}")
                    for g in range(2)
                ]

                def gated(xt, w_dram, by, bg, m):
                    """Embedding GEMM pair + activations; returns ty, tg f32."""
                    wt = wts.tile([128, 2] + kshape, xdt, tag="wt")
                    nc.sync.dma_start(wt[:], w_dram[m])
                    psy = pse.tile([128, BR], F32, tag="psy")
                    psg = pse.tile([128, BR], F32, tag="psg")
                    for br, ps in ((0, psy), (1, psg)):
                        for k in range(KC):
                            if fp8:
                                nc.tensor.matmul(
                                    ps[:], wt[:, br, k], xt(k),
                                    start=(k == 0), stop=(k == KC - 1),
                                    perf_mode=DR,
                                )
                            else:
                                nc.tensor.matmul(
                                    ps[:], wt[:, br, k], xt(k),
                                    start=(k == 0), stop=(k == KC - 1),
                                )
                    ty = tmp.tile([128, BR], F32, tag="ty")
                    nc.scalar.activation(
                        ty[:], psy[:], ACT.Tanh,
                        bias=by[:, m : m + 1], scale=ascale,
                    )
                    tg = tmp.tile([128, BR], F32, tag="tg")
                    nc.scalar.activation(
                        tg[:], psg[:], ACT.Lrelu,
                        bias=bg[:, m : m + 1], scale=ascale, alpha=0.01,
                    )
                    return ty, tg

                # ques embeddings
                for m in range(MC):
                    ty, tg = gated(qt, wq_d, bsb["bqy"], bsb["bqg"], m)
                    nc.vector.scalar_tensor_tensor(
                        qew[:, m, :], ty[:], watt[:, m : m + 1], tg[:],
                        op0=mybir.AluOpType.mult, op1=mybir.AluOpType.mult,
                    )
                    qe = tmp.tile([128, BR], F32, tag="qe")
                    nc.vector.tensor_mul(qe[:], ty[:], tg[:])
                    nc.gpsimd.tensor_mul(qe2[:, m, :], qe[:], qe[:])

                # hist embeddings + num/den accumulation per chunk
                for m in range(MC):
                    ty, tg = gated(ht, wh_d, bsb["bhy"], bsb["bhg"], m)
                    nc.vector.tensor_mul(he[:, m, :], ty[:], tg[:])
                    nc.gpsimd.tensor_mul(he2[:, m, :], he[:, m, :], he[:, m, :])
                    for g in range(2):
                        sl = slice(128 * g, 128 * (g + 1))
                        nc.tensor.matmul(
                            num_ps[g][:], qew[:, m, sl], he[:, m, sl],
                            start=(m == 0), stop=(m == MC - 1),
                        )
                        nc.tensor.matmul(
                            den_ps[g][:], qe2[:, m, sl], he2[:, m, sl],
                            start=(m == 0), stop=(m == MC - 1),
                        )

                # scores + softmax while num/den PSUM is still available;
                # both sqrts issued back-to-back so ACT stays busy while
                # DVE runs the divide/mask chain
                att = []
                rrs = []
                sds = []
                for g in range(2):
                    sd = tmp.tile([128, 128], F32, tag="sd")
                    nc.scalar.activation(sd[:], den_ps[g][:], ACT.Sqrt)
                    sds.append(sd)
                for g in range(2):
                    rd = tmp.tile([128, 128], F32, tag="rd")
                    nc.vector.reciprocal(rd[:], sds[g][:])
                    s = tmp.tile([128, 128], F32, tag="s")
                    nc.vector.tensor_mul(s[:], num_ps[g][:], rd[:])
                    nc.vector.tensor_add(s[:], s[:], mask[:])
                    a = sb.tile([128, 128], F32, name=f"att{g}", tag=f"att{g}")
                    rs = sb.tile([128, 1], F32, name=f"rs{g}", tag=f"rs{g}")
                    nc.scalar.activation(a[:], s[:], ACT.Exp, accum_out=rs[:])
                    r = sb.tile([128, 1], F32, name=f"rrs{g}", tag=f"rrs{g}")
                    nc.vector.reciprocal(r[:], rs[:])
                    att.append(a)
                    rrs.append(r)

            # attention tail + feat
            with (
                tc.tile_pool(name="psa", bufs=2, space="PSUM") as psa,
                tc.tile_pool(name="psf", bufs=4, space="PSUM") as psf,
            ):
                for g in range(2):
                    atp = psa.tile([128, 128], F32, tag="atp")
                    nc.tensor.transpose(atp[:], att[g][:], ident[:])
                    atb = sb.tile([128, 128], BF16, name=f"atb{g}", tag=f"atb{g}")
                    nc.scalar.copy(atb[:], atp[:])
                    fsb = sb.tile([128, IN], BF16, name=f"fsb{g}", tag=f"fsb{g}")
                    for c in range(4):
                        cs = slice(512 * c, 512 * (c + 1))
                        fps = psf.tile([128, 512], F32, tag="fps")
                        nc.tensor.matmul(
                            fps[:], atb[:], hn[:, g, cs], start=True, stop=True
                        )
                        # softmax renorm folded into the PSUM->SBUF copy,
                        # split across DVE / ACT (GpSimd cannot read PSUM)
                        if c % 2 == 0:
                            nc.vector.tensor_scalar_mul(
                                fsb[:, cs], fps[:], rrs[g][:]
                            )
                        else:
                            nc.scalar.activation(
                                fsb[:, cs], fps[:], ACT.Copy, scale=rrs[g][:]
                            )
                        if c % 2 == 1:
                            hs = slice(1024 * (c // 2), 1024 * (c // 2 + 1))
                            eng = nc.sync if (g + c // 2) % 2 == 0 else nc.scalar
                            eng.dma_start(feat_d[g, :, hs], fsb[:, hs])

    _split_multi_waits(nc)
    return nc


# ---------------------------------------------------------------------------
# Host side
# ---------------------------------------------------------------------------

_PROG_CACHE = {}


def _get_prog(fp8):
    if fp8 not in _PROG_CACHE:
        _PROG_CACHE[fp8] = build_program(fp8)
    return _PROG_CACHE[fp8]


def _pack_acts(x, fp8):
    """[BR, IN] -> [128, KC(,2), BR] with k_eff = 256k+128j+p (fp8) or
    128k+p (bf16); contiguous per partition."""
    xt = np.ascontiguousarray(x.T)  # [IN, BR]
    if fp8:
        a = xt.reshape(8, 2, 128, BR).transpose(2, 0, 1, 3)
        return np.ascontiguousarray(np.clip(a, -240, 240)).astype(
            ml_dtypes.float8_e4m3
        )
    a = xt.reshape(16, 128, BR).transpose(1, 0, 2)
    return np.ascontiguousarray(a).astype(ml_dtypes.bfloat16)


def _pack_w(Wy, Wg, fp8):
    """2x[IN, H] -> [MC, 128, 2, KC(,2), 128], scaled for fp8."""
    def one(W):
        if fp8:
            # [k8, j2, p128, m8, h128] -> [m, p, k, j, h]
            a = W.reshape(8, 2, 128, MC, 128).transpose(3, 2, 0, 1, 4)
            a = np.clip(a * WSCALE, -240, 240)
            return a.astype(ml_dtypes.float8_e4m3)
        a = W.reshape(16, 128, MC, 128).transpose(2, 1, 0, 3)
        return a.astype(ml_dtypes.bfloat16)

    y, g = one(Wy), one(Wg)
    return np.ascontiguousarray(np.stack([y, g], axis=2))


def _prep_shared(W_hy, b_hy, W_hg, b_hg, W_qy, b_qy, W_qg, b_qg, W_att, fp8):
    def bvec(b):
        return np.ascontiguousarray(b.reshape(MC, 128).T).astype(np.float32)

    # block-diagonal causal mask over the 4 examples in a 128-row group:
    # 0 where (same example AND h_round <= q_round), NEG elsewhere
    r = np.arange(128)
    same_ex = r[:, None] // 32 == r[None, :] // 32
    causal = (r[None, :] % 32) <= (r[:, None] % 32)
    mask = np.where(same_ex & causal, 0.0, NEG).astype(np.float32)

    return {
        "wh": _pack_w(W_hy, W_hg, fp8),
        "wq": _pack_w(W_qy, W_qg, fp8),
        "bhy": bvec(b_hy),
        "bhg": bvec(b_hg),
        "bqy": bvec(b_qy),
        "bqg": bvec(b_qg),
        "watt": bvec(W_att),
        "mask": np.ascontiguousarray(mask),
        "ident": np.eye(128, dtype=np.float32),
    }


def kernel(
    hist, ques, W_hy, b_hy, W_hg, b_hg, W_qy, b_qy, W_qg, b_qg, W_att, b_att,
    mode="fp8", trace=False,
):
    from concourse.bass_utils import run_bass_kernel_spmd

    fp8 = mode == "fp8"
    hist = np.asarray(hist, np.float32)
    ques = np.asarray(ques, np.float32)
    nc = _get_prog(fp8)
    shared = _prep_shared(
        np.asarray(W_hy, np.float32), np.asarray(b_hy, np.float32),
        np.asarray(W_hg, np.float32), np.asarray(b_hg, np.float32),
        np.asarray(W_qy, np.float32), np.asarray(b_qy, np.float32),
        np.asarray(W_qg, np.float32), np.asarray(b_qg, np.float32),
        np.asarray(W_att, np.float32), fp8,
    )
    in_maps = []
    for c in range(NCORES):
        hs = hist[c * BL : (c + 1) * BL].reshape(BR, IN)
        qs = ques[c * BL : (c + 1) * BL].reshape(BR, IN)
        im = dict(shared)
        im["qt"] = _pack_acts(qs, fp8)
        im["ht"] = _pack_acts(hs, fp8)
        im["hn"] = np.ascontiguousarray(
            hs.reshape(2, 128, IN).transpose(1, 0, 2)
        ).astype(ml_dtypes.bfloat16)
        in_maps.append(im)

    res = run_bass_kernel_spmd(
        nc, in_maps, core_ids=list(range(NCORES)), trace=trace
    )
    feat = np.concatenate(
        [
            r["feat"].astype(np.float32).reshape(BL, R, IN)
            for r in res.results
        ],
        axis=0,
    )
    if trace:
        return feat, res
    return feat


# revision 20
# speedup vs baseline: 1.0121x; 1.0121x over previous
"""Trainium2 Bass kernel for nn_H_ATT (GatedTrans pair-attention block).

Math (per example):
  HE = tanh(hist@W_hy+b_hy) * lrelu(hist@W_hg+b_hg)      [R, H]
  QE = tanh(ques@W_qy+b_qy) * lrelu(ques@W_qg+b_qg)      [R, H]
  num[q,h]  = sum_k QE[q,k]*W_att[k]*HE[h,k]
  den[q,h]  = sqrt(sum_k QE[q,k]^2 * HE[h,k]^2)
  s = num / max(den, eps)          (b_att cancels in softmax)
  att = causal_softmax(s)          (softmax*tril/renorm == masked softmax)
  feat = att @ hist                 [R, 2H]

Sharding: pure data parallel, 8 examples per core on 8 NeuronCores.

The embedding GEMMs dominate both PE time and HBM traffic; they run in
fp8(e4m3) with DoubleRow (weights pre-scaled by 256 on the host, descale
fused into the activation's scale argument). The score/att/feat path stays
bf16/f32. All DRAM inputs are host-packed so every DMA line is contiguous
per partition.
"""

import numpy as np
import ml_dtypes

import bass_rust
import concourse.bass as bass
import concourse.mybir as mybir
import concourse.tile as tile
from concourse.vector_clock import ScopedClock

# ---------------------------------------------------------------------------
# Workaround: this walrus build accepts only ONE semaphore wait on an SP
# Drain, but TileContext's tail drain carries one wait per live semaphore.
# Split them across a chain of drains.
# ---------------------------------------------------------------------------


def _patched_drain_and_barrier(self, tick_clock, wait_clock):
    nc = self.nc
    drain_inst = nc.sync.drain()
    wait_clock.add_sem_waits(
        drain_inst.ins, ScopedClock({None: tick_clock.global_clock})
    )
    waits = list(drain_inst.ins.sync_info.on_wait)
    if len(waits) > 1:
        drain_inst.ins.sync_info = bass_rust.SyncInfo(
            on_wait=waits[:1], on_update=list(drain_inst.ins.sync_info.on_update)
        )
        for i in range(1, len(waits)):
            extra = nc.sync.drain()
            extra.ins.sync_info = bass_rust.SyncInfo(
                on_wait=waits[i : i + 1], on_update=[]
            )
    nc.all_engine_barrier()
    assert self.sems is not None
    popped = nc._tile_sem_poison_stack.pop()
    assert popped is self._sem_poison
    nc.clear_and_free_semaphores(list(self.sems.allocated().values()))
    nc.all_engine_barrier()


tile.TileContext._drain_and_barrier = _patched_drain_and_barrier


def _split_multi_waits(nc):
    """This walrus build accepts at most one semaphore wait per instruction.
    Hoist extra waits onto standalone EventSemaphore instructions inserted
    just before the owning instruction in the same engine's stream."""
    uid = [0]
    for f in nc.m.functions:
        for bb in f.blocks:
            out = []
            for inst in bb.instructions:
                si = inst.sync_info
                if si is not None and len(si.on_wait) > 1:
                    waits = list(si.on_wait)
                    for w in waits[:-1]:
                        nop = mybir.InstEventSemaphore(
                            name=f"I-waitsplit-{uid[0]}", ins=[], outs=[]
                        )
                        uid[0] += 1
                        nop.engine = inst.engine
                        nop.sync_info = bass_rust.SyncInfo(
                            on_wait=[w], on_update=[]
                        )
                        out.append(nop)
                    inst.sync_info = bass_rust.SyncInfo(
                        on_wait=[waits[-1]], on_update=list(si.on_update)
                    )
                out.append(inst)
            bb.instructions[:] = out

# ---------------------------------------------------------------------------

B, R, H, IN = 64, 32, 1024, 2048
NCORES = 8
BL = B // NCORES  # examples per core
BR = BL * R  # 256 rows per core
MC = H // 128  # 8 h chunks
NEG = -1.0e30
WSCALE = 256.0  # fp8 weight pre-scale

F32 = mybir.dt.float32
BF16 = mybir.dt.bfloat16
E4 = mybir.dt.float8e4

ACT = mybir.ActivationFunctionType
DR = mybir.MatmulPerfMode.DoubleRow


def build_program(fp8=True):
    """Per-core Bass program. fp8: embedding GEMMs in e4m3 + DoubleRow;
    else bf16 regular matmuls. Everything downstream is identical."""
    xdt = E4 if fp8 else BF16
    KC = 8 if fp8 else 16  # contraction chunks ([128,2] pairs when fp8)
    kshape = [KC, 2, 128] if fp8 else [KC, 128]
    ascale = 1.0 / WSCALE if fp8 else 1.0

    nc = bass.Bass()
    # activations: [128, KC(,2), BR], partition-major contiguous
    qt_d = nc.dram_tensor("qt", [128] + kshape[:-1] + [BR], xdt, kind="ExternalInput")
    ht_d = nc.dram_tensor("ht", [128] + kshape[:-1] + [BR], xdt, kind="ExternalInput")
    # weights: [MC, 128, 2branch, KC(,2), 128]
    wh_d = nc.dram_tensor("wh", [MC, 128, 2] + kshape, xdt, kind="ExternalInput")
    wq_d = nc.dram_tensor("wq", [MC, 128, 2] + kshape, xdt, kind="ExternalInput")
    hn_d = nc.dram_tensor("hn", [128, 2, IN], BF16, kind="ExternalInput")
    # consts packed: [bqy, bqg, bhy, bhg, watt] along dim1
    cb_d = nc.dram_tensor("cb", [128, 5, MC], F32, kind="ExternalInput")
    # mask + identity packed
    mi_d = nc.dram_tensor("mi", [128, 2, 128], F32, kind="ExternalInput")
    feat_d = nc.dram_tensor("feat", [2, 128, IN], BF16, kind="ExternalOutput")

    with tile.TileContext(nc) as tc:
        with (
            tc.tile_pool(name="sb", bufs=1) as sb,
            tc.tile_pool(name="wts", bufs=4) as wts,
            tc.tile_pool(name="tmp", bufs=2) as tmp,
        ):
            # acts + consts on the ACT hwdge queue (independent of the
            # weight queue so weight-buffer waits never delay them).
            # qt split in two tiles so the first matmuls start earlier.
            KH = KC // 2
            qta = sb.tile([128, KH] + kshape[1:-1] + [BR], xdt, tag="qta")
            nc.scalar.dma_start(qta[:], qt_d[:, :KH])
            qtb = sb.tile([128, KH] + kshape[1:-1] + [BR], xdt, tag="qtb")
            nc.scalar.dma_start(qtb[:], qt_d[:, KH:])

            def qt(k):
                return qta[:, k] if k < KH else qtb[:, k - KH]

            # tiny consts next (first activations need the biases), then
            # the bulk hist loads (not needed until the hist phase / tail)
            cb = sb.tile([128, 5, MC], F32, tag="cb")
            nc.scalar.dma_start(cb[:], cb_d[:])
            bsb = {
                "bqy": cb[:, 0], "bqg": cb[:, 1],
                "bhy": cb[:, 2], "bhg": cb[:, 3],
            }
            watt = cb[:, 4]

            ht_t = sb.tile([128] + kshape[:-1] + [BR], xdt, tag="ht")
            nc.scalar.dma_start(ht_t[:], ht_d[:])

            def ht(k):
                return ht_t[:, k]

            mi = sb.tile([128, 2, 128], F32, tag="mi")
            nc.scalar.dma_start(mi[:], mi_d[:])
            mask = mi[:, 0]
            ident = mi[:, 1]
            hn = sb.tile([128, 2, IN], BF16, tag="hn")
            nc.scalar.dma_start(hn[:], hn_d[:])

            he = sb.tile([128, MC, BR], BF16, tag="he")
            he2 = sb.tile([128, MC, BR], BF16, tag="he2")
            qew = sb.tile([128, MC, BR], BF16, tag="qew")
            qe2 = sb.tile([128, MC, BR], BF16, tag="qe2")

            with (
                tc.tile_pool(name="pse", bufs=3, space="PSUM") as pse,
                tc.tile_pool(name="psnd", bufs=1, space="PSUM") as psnd,
            ):
                num_t = psnd.tile([128, 256], F32, name="num", tag="num")
                den_t = psnd.tile([128, 256], F32, name="den", tag="den")
                num_ps = [num_t[:, 128 * g : 128 * (g + 1)] for g in range(2)]
                den_ps = [den_t[:, 128 * g : 128 * (g + 1)] for g in range(2)]

                def gated(xt, w_dram, by, bg, m):
                    """Embedding GEMM pair + activations; returns ty, tg f32."""
                    wt = wts.tile([128, 2] + kshape, xdt, tag="wt")
                    nc.sync.dma_start(wt[:], w_dram[m])
                    psy = pse.tile([128, BR], F32, tag="psy")
                    psg = pse.tile([128, BR], F32, tag="psg")
                    for br, ps in ((0, psy), (1, psg)):
                        for k in range(KC):
                            if fp8:
                                nc.tensor.matmul(
                                    ps[:], wt[:, br, k], xt(k),
                                    start=(k == 0), stop=(k == KC - 1),
                                    perf_mode=DR,
                                )
                            else:
                                nc.tensor.matmul(
                                    ps[:], wt[:, br, k], xt(k),
                                    start=(k == 0), stop=(k == KC - 1),
                                )
                    ty = tmp.tile([128, BR], F32, tag="ty")
                    nc.scalar.activation(
                        ty[:], psy[:], ACT.Tanh,
                        bias=by[:, m : m + 1], scale=ascale,
                    )
                    tg = tmp.tile([128, BR], F32, tag="tg")
                    nc.scalar.activation(
                        tg[:], psg[:], ACT.Lrelu,
                        bias=bg[:, m : m + 1], scale=ascale, alpha=0.01,
                    )
                    return ty, tg

                # ques embeddings
                for m in range(MC):
                    ty, tg = gated(qt, wq_d, bsb["bqy"], bsb["bqg"], m)
                    nc.vector.scalar_tensor_tensor(
                        qew[:, m, :], ty[:], watt[:, m : m + 1], tg[:],
                        op0=mybir.AluOpType.mult, op1=mybir.AluOpType.mult,
                    )
                    qe = tmp.tile([128, BR], F32, tag="qe")
                    nc.vector.tensor_mul(qe[:], ty[:], tg[:])
                    nc.gpsimd.tensor_mul(qe2[:, m, :], qe[:], qe[:])

                # hist embeddings + num/den accumulation per chunk
                for m in range(MC):
                    ty, tg = gated(ht, wh_d, bsb["bhy"], bsb["bhg"], m)
                    nc.vector.tensor_mul(he[:, m, :], ty[:], tg[:])
                    nc.gpsimd.tensor_mul(he2[:, m, :], he[:, m, :], he[:, m, :])
                    for g in range(2):
                        sl = slice(128 * g, 128 * (g + 1))
                        nc.tensor.matmul(
                            num_ps[g], qew[:, m, sl], he[:, m, sl],
                            start=(m == 0), stop=(m == MC - 1),
                        )
                        nc.tensor.matmul(
                            den_ps[g], qe2[:, m, sl], he2[:, m, sl],
                            start=(m == 0), stop=(m == MC - 1),
                        )

                # scores + softmax while num/den PSUM is still available;
                # both sqrts issued back-to-back so ACT stays busy while
                # DVE runs the divide/mask chain
                att = []
                rrs = []
                sds = []
                for g in range(2):
                    sd = tmp.tile([128, 128], F32, tag="sd")
                    nc.scalar.activation(sd[:], den_ps[g], ACT.Sqrt)
                    sds.append(sd)
                for g in range(2):
                    rd = tmp.tile([128, 128], F32, tag="rd")
                    nc.vector.reciprocal(rd[:], sds[g][:])
                    s = tmp.tile([128, 128], F32, tag="s")
                    nc.vector.tensor_mul(s[:], num_ps[g], rd[:])
                    nc.vector.tensor_add(s[:], s[:], mask)
                    a = sb.tile([128, 128], F32, name=f"att{g}", tag=f"att{g}")
                    rs = sb.tile([128, 1], F32, name=f"rs{g}", tag=f"rs{g}")
                    nc.scalar.activation(a[:], s[:], ACT.Exp, accum_out=rs[:])
                    r = sb.tile([128, 1], F32, name=f"rrs{g}", tag=f"rrs{g}")
                    nc.vector.reciprocal(r[:], rs[:])
                    att.append(a)
                    rrs.append(r)

            # attention tail + feat
            with (
                tc.tile_pool(name="psa", bufs=2, space="PSUM") as psa,
                tc.tile_pool(name="psf", bufs=4, space="PSUM") as psf,
            ):
                for g in range(2):
                    atp = psa.tile([128, 128], F32, tag="atp")
                    nc.tensor.transpose(atp[:], att[g][:], ident)
                    atb = sb.tile([128, 128], BF16, name=f"atb{g}", tag=f"atb{g}")
                    nc.scalar.copy(atb[:], atp[:])
                    fsb = sb.tile([128, IN], BF16, name=f"fsb{g}", tag=f"fsb{g}")
                    for c in range(4):
                        cs = slice(512 * c, 512 * (c + 1))
                        fps = psf.tile([128, 512], F32, tag="fps")
                        nc.tensor.matmul(
                            fps[:], atb[:], hn[:, g, cs], start=True, stop=True
                        )
                        # softmax renorm folded into the PSUM->SBUF copy,
                        # split across DVE / ACT (GpSimd cannot read PSUM)
                        if c % 2 == 0:
                            nc.vector.tensor_scalar_mul(
                                fsb[:, cs], fps[:], rrs[g][:]
                            )
                        else:
                            nc.scalar.activation(
                                fsb[:, cs], fps[:], ACT.Copy, scale=rrs[g][:]
                            )
                        if c % 2 == 1:
                            hs = slice(1024 * (c // 2), 1024 * (c // 2 + 1))
                            eng = nc.sync if (g + c // 2) % 2 == 0 else nc.scalar
                            eng.dma_start(feat_d[g, :, hs], fsb[:, hs])

    _split_multi_waits(nc)
    return nc


# ---------------------------------------------------------------------------
# Host side
# ---------------------------------------------------------------------------

_PROG_CACHE = {}


def _get_prog(fp8):
    if fp8 not in _PROG_CACHE:
        _PROG_CACHE[fp8] = build_program(fp8)
    return _PROG_CACHE[fp8]


def _pack_acts(x, fp8):
    """[BR, IN] -> [128, KC(,2), BR] with k_eff = 256k+128j+p (fp8) or
    128k+p (bf16); contiguous per partition."""
    xt = np.ascontiguousarray(x.T)  # [IN, BR]
    if fp8:
        a = xt.reshape(8, 2, 128, BR).transpose(2, 0, 1, 3)
        return np.ascontiguousarray(np.clip(a, -240, 240)).astype(
            ml_dtypes.float8_e4m3
        )
    a = xt.reshape(16, 128, BR).transpose(1, 0, 2)
    return np.ascontiguousarray(a).astype(ml_dtypes.bfloat16)


def _pack_w(Wy, Wg, fp8):
    """2x[IN, H] -> [MC, 128, 2, KC(,2), 128], scaled for fp8."""
    def one(W):
        if fp8:
            # [k8, j2, p128, m8, h128] -> [m, p, k, j, h]
            a = W.reshape(8, 2, 128, MC, 128).transpose(3, 2, 0, 1, 4)
            a = np.clip(a * WSCALE, -240, 240)
            return a.astype(ml_dtypes.float8_e4m3)
        a = W.reshape(16, 128, MC, 128).transpose(2, 1, 0, 3)
        return a.astype(ml_dtypes.bfloat16)

    y, g = one(Wy), one(Wg)
    return np.ascontiguousarray(np.stack([y, g], axis=2))


def _prep_shared(W_hy, b_hy, W_hg, b_hg, W_qy, b_qy, W_qg, b_qg, W_att, fp8):
    def bvec(b):
        return np.ascontiguousarray(b.reshape(MC, 128).T).astype(np.float32)

    # block-diagonal causal mask over the 4 examples in a 128-row group:
    # 0 where (same example AND h_round <= q_round), NEG elsewhere
    r = np.arange(128)
    same_ex = r[:, None] // 32 == r[None, :] // 32
    causal = (r[None, :] % 32) <= (r[:, None] % 32)
    mask = np.where(same_ex & causal, 0.0, NEG).astype(np.float32)

    cb = np.stack(
        [bvec(b_qy), bvec(b_qg), bvec(b_hy), bvec(b_hg), bvec(W_att)], axis=1
    )
    mi = np.stack([mask, np.eye(128, dtype=np.float32)], axis=1)
    return {
        "wh": _pack_w(W_hy, W_hg, fp8),
        "wq": _pack_w(W_qy, W_qg, fp8),
        "cb": np.ascontiguousarray(cb),
        "mi": np.ascontiguousarray(mi),
    }


def kernel(
    hist, ques, W_hy, b_hy, W_hg, b_hg, W_qy, b_qy, W_qg, b_qg, W_att, b_att,
    mode="fp8", trace=False,
):
    from concourse.bass_utils import run_bass_kernel_spmd

    fp8 = mode == "fp8"
    hist = np.asarray(hist, np.float32)
    ques = np.asarray(ques, np.float32)
    nc = _get_prog(fp8)
    shared = _prep_shared(
        np.asarray(W_hy, np.float32), np.asarray(b_hy, np.float32),
        np.asarray(W_hg, np.float32), np.asarray(b_hg, np.float32),
        np.asarray(W_qy, np.float32), np.asarray(b_qy, np.float32),
        np.asarray(W_qg, np.float32), np.asarray(b_qg, np.float32),
        np.asarray(W_att, np.float32), fp8,
    )
    in_maps = []
    for c in range(NCORES):
        hs = hist[c * BL : (c + 1) * BL].reshape(BR, IN)
        qs = ques[c * BL : (c + 1) * BL].reshape(BR, IN)
        im = dict(shared)
        im["qt"] = _pack_acts(qs, fp8)
        im["ht"] = _pack_acts(hs, fp8)
        im["hn"] = np.ascontiguousarray(
            hs.reshape(2, 128, IN).transpose(1, 0, 2)
        ).astype(ml_dtypes.bfloat16)
        in_maps.append(im)

    res = run_bass_kernel_spmd(
        nc, in_maps, core_ids=list(range(NCORES)), trace=trace
    )
    feat = np.concatenate(
        [
            r["feat"].astype(np.float32).reshape(BL, R, IN)
            for r in res.results
        ],
        axis=0,
    )
    if trace:
        return feat, res
    return feat


# revision 21
# speedup vs baseline: 1.0193x; 1.0072x over previous
"""Trainium2 Bass kernel for nn_H_ATT (GatedTrans pair-attention block).

Math (per example):
  HE = tanh(hist@W_hy+b_hy) * lrelu(hist@W_hg+b_hg)      [R, H]
  QE = tanh(ques@W_qy+b_qy) * lrelu(ques@W_qg+b_qg)      [R, H]
  num[q,h]  = sum_k QE[q,k]*W_att[k]*HE[h,k]
  den[q,h]  = sqrt(sum_k QE[q,k]^2 * HE[h,k]^2)
  s = num / max(den, eps)          (b_att cancels in softmax)
  att = causal_softmax(s)          (softmax*tril/renorm == masked softmax)
  feat = att @ hist                 [R, 2H]

Sharding: pure data parallel, 8 examples per core on 8 NeuronCores.

The embedding GEMMs dominate both PE time and HBM traffic; they run in
fp8(e4m3) with DoubleRow (weights pre-scaled by 256 on the host, descale
fused into the activation's scale argument). The score/att/feat path stays
bf16/f32. All DRAM inputs are host-packed so every DMA line is contiguous
per partition.
"""

import numpy as np
import ml_dtypes

import bass_rust
import concourse.bass as bass
import concourse.mybir as mybir
import concourse.tile as tile
from concourse.vector_clock import ScopedClock

# ---------------------------------------------------------------------------
# Workaround: this walrus build accepts only ONE semaphore wait on an SP
# Drain, but TileContext's tail drain carries one wait per live semaphore.
# Split them across a chain of drains.
# ---------------------------------------------------------------------------


def _patched_drain_and_barrier(self, tick_clock, wait_clock):
    nc = self.nc
    drain_inst = nc.sync.drain()
    wait_clock.add_sem_waits(
        drain_inst.ins, ScopedClock({None: tick_clock.global_clock})
    )
    waits = list(drain_inst.ins.sync_info.on_wait)
    if len(waits) > 1:
        drain_inst.ins.sync_info = bass_rust.SyncInfo(
            on_wait=waits[:1], on_update=list(drain_inst.ins.sync_info.on_update)
        )
        for i in range(1, len(waits)):
            extra = nc.sync.drain()
            extra.ins.sync_info = bass_rust.SyncInfo(
                on_wait=waits[i : i + 1], on_update=[]
            )
    nc.all_engine_barrier()
    assert self.sems is not None
    popped = nc._tile_sem_poison_stack.pop()
    assert popped is self._sem_poison
    nc.clear_and_free_semaphores(list(self.sems.allocated().values()))
    nc.all_engine_barrier()


tile.TileContext._drain_and_barrier = _patched_drain_and_barrier


def _split_multi_waits(nc):
    """This walrus build accepts at most one semaphore wait per instruction.
    Hoist extra waits onto standalone EventSemaphore instructions inserted
    just before the owning instruction in the same engine's stream."""
    uid = [0]
    for f in nc.m.functions:
        for bb in f.blocks:
            out = []
            for inst in bb.instructions:
                si = inst.sync_info
                if si is not None and len(si.on_wait) > 1:
                    waits = list(si.on_wait)
                    for w in waits[:-1]:
                        nop = mybir.InstEventSemaphore(
                            name=f"I-waitsplit-{uid[0]}", ins=[], outs=[]
                        )
                        uid[0] += 1
                        nop.engine = inst.engine
                        nop.sync_info = bass_rust.SyncInfo(
                            on_wait=[w], on_update=[]
                        )
                        out.append(nop)
                    inst.sync_info = bass_rust.SyncInfo(
                        on_wait=[waits[-1]], on_update=list(si.on_update)
                    )
                out.append(inst)
            bb.instructions[:] = out

# ---------------------------------------------------------------------------

B, R, H, IN = 64, 32, 1024, 2048
NCORES = 8
BL = B // NCORES  # examples per core
BR = BL * R  # 256 rows per core
MC = H // 128  # 8 h chunks
NEG = -1.0e30
WSCALE = 256.0  # fp8 weight pre-scale

F32 = mybir.dt.float32
BF16 = mybir.dt.bfloat16
E4 = mybir.dt.float8e4

ACT = mybir.ActivationFunctionType
DR = mybir.MatmulPerfMode.DoubleRow


def build_program(fp8=True):
    """Per-core Bass program. fp8: embedding GEMMs in e4m3 + DoubleRow;
    else bf16 regular matmuls. Everything downstream is identical."""
    xdt = E4 if fp8 else BF16
    KC = 8 if fp8 else 16  # contraction chunks ([128,2] pairs when fp8)
    kshape = [KC, 2, 128] if fp8 else [KC, 128]
    ascale = 1.0 / WSCALE if fp8 else 1.0

    nc = bass.Bass()
    # activations: [128, KC(,2), BR], partition-major contiguous
    qt_d = nc.dram_tensor("qt", [128] + kshape[:-1] + [BR], xdt, kind="ExternalInput")
    ht_d = nc.dram_tensor("ht", [128] + kshape[:-1] + [BR], xdt, kind="ExternalInput")
    # weights: [MC, 128, 2branch, KC(,2), 128]
    wh_d = nc.dram_tensor("wh", [MC, 128, 2] + kshape, xdt, kind="ExternalInput")
    wq_d = nc.dram_tensor("wq", [MC, 128, 2] + kshape, xdt, kind="ExternalInput")
    hn_d = nc.dram_tensor("hn", [128, 2, IN], BF16, kind="ExternalInput")
    # consts packed: [bqy, bqg, bhy, bhg, watt] along dim1
    cb_d = nc.dram_tensor("cb", [128, 5, MC], F32, kind="ExternalInput")
    # mask + identity packed
    mi_d = nc.dram_tensor("mi", [128, 2, 128], F32, kind="ExternalInput")
    feat_d = nc.dram_tensor("feat", [2, 128, IN], BF16, kind="ExternalOutput")

    with tile.TileContext(nc) as tc:
        with (
            tc.tile_pool(name="sb", bufs=1) as sb,
            tc.tile_pool(name="wts", bufs=4) as wts,
            tc.tile_pool(name="tmp", bufs=2) as tmp,
        ):
            # acts + consts on the ACT hwdge queue (independent of the
            # weight queue so weight-buffer waits never delay them).
            # qt split in two tiles so the first matmuls start earlier.
            KH = KC // 2
            qta = sb.tile([128, KH] + kshape[1:-1] + [BR], xdt, tag="qta")
            nc.scalar.dma_start(qta[:], qt_d[:, :KH])
            qtb = sb.tile([128, KH] + kshape[1:-1] + [BR], xdt, tag="qtb")
            nc.scalar.dma_start(qtb[:], qt_d[:, KH:])

            def qt(k):
                return qta[:, k] if k < KH else qtb[:, k - KH]

            # tiny consts next (first activations need the biases), then
            # the bulk hist loads (not needed until the hist phase / tail)
            cb = sb.tile([128, 5, MC], F32, tag="cb")
            nc.scalar.dma_start(cb[:], cb_d[:])
            bsb = {
                "bqy": cb[:, 0], "bqg": cb[:, 1],
                "bhy": cb[:, 2], "bhg": cb[:, 3],
            }
            watt = cb[:, 4]

            ht_t = sb.tile([128] + kshape[:-1] + [BR], xdt, tag="ht")
            nc.scalar.dma_start(ht_t[:], ht_d[:])

            def ht(k):
                return ht_t[:, k]

            mi = sb.tile([128, 2, 128], F32, tag="mi")
            nc.scalar.dma_start(mi[:], mi_d[:])
            mask = mi[:, 0]
            ident = mi[:, 1]
            hn = sb.tile([128, 2, IN], BF16, tag="hn")
            nc.scalar.dma_start(hn[:], hn_d[:])

            he = sb.tile([128, MC, BR], BF16, tag="he")
            he2 = sb.tile([128, MC, BR], BF16, tag="he2")
            qew = sb.tile([128, MC, BR], BF16, tag="qew")
            qe2 = sb.tile([128, MC, BR], BF16, tag="qe2")

            with (
                tc.tile_pool(name="pse", bufs=2, space="PSUM") as pse,
                tc.tile_pool(name="psnd", bufs=1, space="PSUM") as psnd,
            ):
                num_t = psnd.tile([128, 256], F32, name="num", tag="num")
                den_t = psnd.tile([128, 256], F32, name="den", tag="den")
                num_ps = [num_t[:, 128 * g : 128 * (g + 1)] for g in range(2)]
                den_ps = [den_t[:, 128 * g : 128 * (g + 1)] for g in range(2)]

                def gated(xt, w_dram, by, bg, m):
                    """Embedding GEMM pair + activations; returns ty, tg f32."""
                    wt = wts.tile([128, 2] + kshape, xdt, tag="wt")
                    nc.sync.dma_start(wt[:], w_dram[m])
                    psy = pse.tile([128, BR], F32, tag="psy")
                    psg = pse.tile([128, BR], F32, tag="psg")
                    for br, ps in ((0, psy), (1, psg)):
                        for k in range(KC):
                            if fp8:
                                nc.tensor.matmul(
                                    ps[:], wt[:, br, k], xt(k),
                                    start=(k == 0), stop=(k == KC - 1),
                                    perf_mode=DR,
                                )
                            else:
                                nc.tensor.matmul(
                                    ps[:], wt[:, br, k], xt(k),
                                    start=(k == 0), stop=(k == KC - 1),
                                )
                    ty = tmp.tile([128, BR], F32, tag="ty")
                    nc.scalar.activation(
                        ty[:], psy[:], ACT.Tanh,
                        bias=by[:, m : m + 1], scale=ascale,
                    )
                    tg = tmp.tile([128, BR], F32, tag="tg")
                    nc.scalar.activation(
                        tg[:], psg[:], ACT.Lrelu,
                        bias=bg[:, m : m + 1], scale=ascale, alpha=0.01,
                    )
                    return ty, tg

                # ques embeddings
                for m in range(MC):
                    ty, tg = gated(qt, wq_d, bsb["bqy"], bsb["bqg"], m)
                    nc.vector.scalar_tensor_tensor(
                        qew[:, m, :], ty[:], watt[:, m : m + 1], tg[:],
                        op0=mybir.AluOpType.mult, op1=mybir.AluOpType.mult,
                    )
                    qe = tmp.tile([128, BR], F32, tag="qe")
                    nc.vector.tensor_mul(qe[:], ty[:], tg[:])
                    nc.gpsimd.tensor_mul(qe2[:, m, :], qe[:], qe[:])

                # hist embeddings + num/den accumulation per chunk
                for m in range(MC):
                    ty, tg = gated(ht, wh_d, bsb["bhy"], bsb["bhg"], m)
                    nc.vector.tensor_mul(he[:, m, :], ty[:], tg[:])
                    nc.gpsimd.tensor_mul(he2[:, m, :], he[:, m, :], he[:, m, :])
                    for g in range(2):
                        sl = slice(128 * g, 128 * (g + 1))
                        nc.tensor.matmul(
                            num_ps[g], qew[:, m, sl], he[:, m, sl],
                            start=(m == 0), stop=(m == MC - 1),
                        )
                        nc.tensor.matmul(
                            den_ps[g], qe2[:, m, sl], he2[:, m, sl],
                            start=(m == 0), stop=(m == MC - 1),
                        )

                # scores + softmax while num/den PSUM is still available;
                # both sqrts issued back-to-back so ACT stays busy while
                # DVE runs the divide/mask chain
                att = []
                rrs = []
                sds = []
                for g in range(2):
                    sd = tmp.tile([128, 128], F32, tag="sd")
                    nc.scalar.activation(sd[:], den_ps[g], ACT.Sqrt)
                    sds.append(sd)
                for g in range(2):
                    rd = tmp.tile([128, 128], F32, tag="rd")
                    nc.vector.reciprocal(rd[:], sds[g][:])
                    s = tmp.tile([128, 128], F32, tag="s")
                    nc.vector.tensor_mul(s[:], num_ps[g], rd[:])
                    nc.vector.tensor_add(s[:], s[:], mask)
                    a = sb.tile([128, 128], F32, name=f"att{g}", tag=f"att{g}")
                    rs = sb.tile([128, 1], F32, name=f"rs{g}", tag=f"rs{g}")
                    nc.scalar.activation(a[:], s[:], ACT.Exp, accum_out=rs[:])
                    r = sb.tile([128, 1], F32, name=f"rrs{g}", tag=f"rrs{g}")
                    nc.vector.reciprocal(r[:], rs[:])
                    att.append(a)
                    rrs.append(r)

            # attention tail + feat
            with (
                tc.tile_pool(name="psa", bufs=2, space="PSUM") as psa,
                tc.tile_pool(name="psf", bufs=4, space="PSUM") as psf,
            ):
                for g in range(2):
                    atp = psa.tile([128, 128], F32, tag="atp")
                    nc.tensor.transpose(atp[:], att[g][:], ident)
                    atb = sb.tile([128, 128], BF16, name=f"atb{g}", tag=f"atb{g}")
                    nc.scalar.copy(atb[:], atp[:])
                    fsb = sb.tile([128, IN], BF16, name=f"fsb{g}", tag=f"fsb{g}")
                    for c in range(4):
                        cs = slice(512 * c, 512 * (c + 1))
                        fps = psf.tile([128, 512], F32, tag="fps")
                        nc.tensor.matmul(
                            fps[:], atb[:], hn[:, g, cs], start=True, stop=True
                        )
                        # softmax renorm folded into the PSUM->SBUF copy,
                        # split across DVE / ACT (GpSimd cannot read PSUM)
                        if c % 2 == 0:
                            nc.vector.tensor_scalar_mul(
                                fsb[:, cs], fps[:], rrs[g][:]
                            )
                        else:
                            nc.scalar.activation(
                                fsb[:, cs], fps[:], ACT.Copy, scale=rrs[g][:]
                            )
                        if c % 2 == 1:
                            hs = slice(1024 * (c // 2), 1024 * (c // 2 + 1))
                            eng = nc.sync if (g + c // 2) % 2 == 0 else nc.scalar
                            eng.dma_start(feat_d[g, :, hs], fsb[:, hs])

    _split_multi_waits(nc)
    return nc


# ---------------------------------------------------------------------------
# Host side
# ---------------------------------------------------------------------------

_PROG_CACHE = {}


def _get_prog(fp8):
    if fp8 not in _PROG_CACHE:
        _PROG_CACHE[fp8] = build_program(fp8)
    return _PROG_CACHE[fp8]


def _pack_acts(x, fp8):
    """[BR, IN] -> [128, KC(,2), BR] with k_eff = 256k+128j+p (fp8) or
    128k+p (bf16); contiguous per partition."""
    xt = np.ascontiguousarray(x.T)  # [IN, BR]
    if fp8:
        a = xt.reshape(8, 2, 128, BR).transpose(2, 0, 1, 3)
        return np.ascontiguousarray(np.clip(a, -240, 240)).astype(
            ml_dtypes.float8_e4m3
        )
    a = xt.reshape(16, 128, BR).transpose(1, 0, 2)
    return np.ascontiguousarray(a).astype(ml_dtypes.bfloat16)


def _pack_w(Wy, Wg, fp8):
    """2x[IN, H] -> [MC, 128, 2, KC(,2), 128], scaled for fp8."""
    def one(W):
        if fp8:
            # [k8, j2, p128, m8, h128] -> [m, p, k, j, h]
            a = W.reshape(8, 2, 128, MC, 128).transpose(3, 2, 0, 1, 4)
            a = np.clip(a * WSCALE, -240, 240)
            return a.astype(ml_dtypes.float8_e4m3)
        a = W.reshape(16, 128, MC, 128).transpose(2, 1, 0, 3)
        return a.astype(ml_dtypes.bfloat16)

    y, g = one(Wy), one(Wg)
    return np.ascontiguousarray(np.stack([y, g], axis=2))


def _prep_shared(W_hy, b_hy, W_hg, b_hg, W_qy, b_qy, W_qg, b_qg, W_att, fp8):
    def bvec(b):
        return np.ascontiguousarray(b.reshape(MC, 128).T).astype(np.float32)

    # block-diagonal causal mask over the 4 examples in a 128-row group:
    # 0 where (same example AND h_round <= q_round), NEG elsewhere
    r = np.arange(128)
    same_ex = r[:, None] // 32 == r[None, :] // 32
    causal = (r[None, :] % 32) <= (r[:, None] % 32)
    mask = np.where(same_ex & causal, 0.0, NEG).astype(np.float32)

    cb = np.stack(
        [bvec(b_qy), bvec(b_qg), bvec(b_hy), bvec(b_hg), bvec(W_att)], axis=1
    )
    mi = np.stack([mask, np.eye(128, dtype=np.float32)], axis=1)
    return {
        "wh": _pack_w(W_hy, W_hg, fp8),
        "wq": _pack_w(W_qy, W_qg, fp8),
        "cb": np.ascontiguousarray(cb),
        "mi": np.ascontiguousarray(mi),
    }


def kernel(
    hist, ques, W_hy, b_hy, W_hg, b_hg, W_qy, b_qy, W_qg, b_qg, W_att, b_att,
    mode="fp8", trace=False,
):
    from concourse.bass_utils import run_bass_kernel_spmd

    fp8 = mode == "fp8"
    hist = np.asarray(hist, np.float32)
    ques = np.asarray(ques, np.float32)
    nc = _get_prog(fp8)
    shared = _prep_shared(
        np.asarray(W_hy, np.float32), np.asarray(b_hy, np.float32),
        np.asarray(W_hg, np.float32), np.asarray(b_hg, np.float32),
        np.asarray(W_qy, np.float32), np.asarray(b_qy, np.float32),
        np.asarray(W_qg, np.float32), np.asarray(b_qg, np.float32),
        np.asarray(W_att, np.float32), fp8,
    )
    in_maps = []
    for c in range(NCORES):
        hs = hist[c * BL : (c + 1) * BL].reshape(BR, IN)
        qs = ques[c * BL : (c + 1) * BL].reshape(BR, IN)
        im = dict(shared)
        im["qt"] = _pack_acts(qs, fp8)
        im["ht"] = _pack_acts(hs, fp8)
        im["hn"] = np.ascontiguousarray(
            hs.reshape(2, 128, IN).transpose(1, 0, 2)
        ).astype(ml_dtypes.bfloat16)
        in_maps.append(im)

    res = run_bass_kernel_spmd(
        nc, in_maps, core_ids=list(range(NCORES)), trace=trace
    )
    feat = np.concatenate(
        [
            r["feat"].astype(np.float32).reshape(BL, R, IN)
            for r in res.results
        ],
        axis=0,
    )
    if trace:
        return feat, res
    return feat
